# revision 1
# baseline (speedup 1.0000x reference)
"""DeformConv2d (offset conv + bilinear deformable conv) on 8 trn2 NeuronCores.

Sharding: data-parallel over (batch, H-half): core = 2*b + h handles batch b,
output rows [64h, 64h+64).

Per-core device pipeline:
  1. offset conv (PE, fp32): 9 accumulating shifted matmuls per 4-row tile,
     dy taps in PSUM rows 0-8, dx taps in rows 32-40 (weight columns padded)
  2. coordinate math (DVE, fp32) on two [128, 2048] planes (Y and X); tap row
     = 32*g + kk. Fraction via tensor_scalar python_mod; clamp; patch index.
  3. per (tile, tap): idx row is bounced through DRAM to produce the
     16-partition-wrapped index layout, then one dma_gather pulls 512
     512-byte 2x2xC patches from the HBM patch table ->
     G [128=(4 neighbors x 64ch), 2, 512] bf16
  4. the 4 bilinear weight rows are broadcast across channel partitions with
     one-hot selector matmuls (PE) and applied with one dense bf16 multiply
  5. per tap 2 accumulating matmuls (lhsT = [W;W]) sum the 4 weighted
     neighbors and contract channels into PSUM [64, 512]
  6. bias add (ACT) and store.

Numerics: patch table / gather / main matmul in bf16; everything else fp32.
"""

import numpy as np
import ml_dtypes

import concourse.bass as bass
import concourse.bacc as bacc
import concourse.tile as tile
import concourse.mybir as mybir
from concourse.bass_utils import run_bass_kernel_spmd

B, C, H, W, O = 4, 64, 128, 128, 64
K = 3
# padded image [131, 131]: rows/cols 0,1 zero; 2..129 data; 130 zero.
PH = PW = H + 3
NPOS = PH * PW  # 17161 patch-table entries (< int16 max)
HALF = H // 2  # output rows per core
NT = HALF // 4  # 16 tiles of 4 rows x 128 cols
NI = 512  # pixels (gather descriptors) per (tile, tap)
NW = NI // 16  # wrapped idx columns per tap
TAPS = K * K
XROWS = HALF + 3  # padded rows a core needs for its conv
MC = 41  # conv PSUM rows: dy 0-8, dx 32-40
F32 = mybir.dt.float32
BF16 = mybir.dt.bfloat16
I16 = mybir.dt.int16

AluOp = mybir.AluOpType


def _build_program():
    nc = bacc.Bacc("TRN2", target_bir_lowering=False, debug=False)

    xs_d = nc.dram_tensor("xs", [C, XROWS * PW], F32, kind="ExternalInput")
    table_d = nc.dram_tensor("table", [NPOS, 4 * C], BF16, kind="ExternalInput")
    basey_d = nc.dram_tensor("basey", [128, 2048], F32, kind="ExternalInput")
    basex_d = nc.dram_tensor("basex", [128, 2048], F32, kind="ExternalInput")
    woff_d = nc.dram_tensor("woff", [C, TAPS * MC], F32, kind="ExternalInput")
    wdcn_d = nc.dram_tensor("wdcn", [2 * C, TAPS * O], BF16, kind="ExternalInput")
    sel_d = nc.dram_tensor("sel", [128, 36 * O], BF16, kind="ExternalInput")
    bias_d = nc.dram_tensor("bias", [O, 1], F32, kind="ExternalInput")
    out_d = nc.dram_tensor("outc", [O, HALF * W], F32, kind="ExternalOutput")

    with tile.TileContext(nc) as tc:
        with (
            tc.tile_pool(name="const", bufs=1) as cp,
            tc.tile_pool(name="coord", bufs=1) as co,
            tc.tile_pool(name="gbuf", bufs=6) as gp,
            tc.tile_pool(name="wibuf", bufs=4) as wp,
            tc.tile_pool(name="scb", bufs=4) as sp,
            tc.tile_pool(name="idxp", bufs=4) as ip,
            tc.tile_pool(name="outp", bufs=3) as op_,
            tc.tile_pool(name="dram", bufs=1, space="DRAM") as dp,
            tc.tile_pool(name="ps_off", bufs=2, space="PSUM") as ps_off,
            tc.tile_pool(name="ps_w", bufs=2, space="PSUM") as ps_w,
            tc.tile_pool(name="ps_out", bufs=2, space="PSUM") as ps_out,
        ):
            xs = cp.tile([C, XROWS * PW], F32)
            nc.sync.dma_start(out=xs[:], in_=xs_d[:])
            basey = cp.tile([128, 2048], F32)
            nc.sync.dma_start(out=basey[:], in_=basey_d[:])
            basex = cp.tile([128, 2048], F32)
            nc.sync.dma_start(out=basex[:], in_=basex_d[:])
            woff = cp.tile([C, TAPS * MC], F32)
            nc.sync.dma_start(out=woff[:], in_=woff_d[:])
            wdcn = cp.tile([2 * C, TAPS * O], BF16)
            nc.sync.dma_start(out=wdcn[:], in_=wdcn_d[:])
            sel = cp.tile([128, 36 * O], BF16)
            nc.sync.dma_start(out=sel[:], in_=sel_d[:])
            bias = cp.tile([O, 1], F32)
            nc.sync.dma_start(out=bias[:], in_=bias_d[:])

            # ---- 1. offset conv -> coordY / coordX planes ----
            coordY = co.tile([128, 2048], F32, tag="cy")
            coordX = co.tile([128, 2048], F32, tag="cx")
            nc.vector.memset(coordY[:], 0.0)
            nc.vector.memset(coordX[:], 0.0)
            for t in range(NT):
                g, s = t // 4, t % 4
                po = ps_off.tile([MC, NI], F32)
                for kk in range(TAPS):
                    ki, kj = kk // K, kk % K
                    # pixels streamed in plane order f: pixel q(f) =
                    # (f%32)*16 + f//32 = k*128 + m*16 + a for f=(a,k,m)
                    xap = xs[:]
                    rhs = bass.AP(
                        tensor=xap.tensor,
                        offset=xap.offset + (4 * t + ki + 1) * PW + 1 + kj,
                        ap=[xap.ap[0], [1, 16], [PW, 4], [16, 8]],
                    )
                    nc.tensor.matmul(
                        po[:],
                        woff[:, kk * MC : (kk + 1) * MC],
                        rhs,
                        start=(kk == 0),
                        stop=(kk == TAPS - 1),
                    )
                nc.scalar.copy(
                    coordY[32 * g : 32 * g + 9, s * NI : (s + 1) * NI], po[0:9, :]
                )
                nc.scalar.copy(
                    coordX[32 * g : 32 * g + 9, s * NI : (s + 1) * NI], po[32:41, :]
                )

            # ---- 2. coordinate math (full-plane ops, slot-reusing tags) ----
            Py = co.tile([128, 2048], F32, tag="py")
            nc.vector.tensor_tensor(Py[:], coordY[:], basey[:], AluOp.add)
            Px = co.tile([128, 2048], F32, tag="px")
            nc.vector.tensor_tensor(Px[:], coordX[:], basex[:], AluOp.add)
            # exact floor: p in [512, 1024) -> clear the 14 low mantissa
            # bits of the f32 encoding (integer part only)
            Fy = co.tile([128, 2048], F32, tag="cy")  # reuse coordY slot
            nc.vector.tensor_scalar(
                Fy[:].bitcast(mybir.dt.int32), Py[:].bitcast(mybir.dt.int32),
                -16384, None, AluOp.bitwise_and,
            )
            Fx = co.tile([128, 2048], F32, tag="cx")
            nc.vector.tensor_scalar(
                Fx[:].bitcast(mybir.dt.int32), Px[:].bitcast(mybir.dt.int32),
                -16384, None, AluOp.bitwise_and,
            )
            Ly = co.tile([128, 2048], F32, tag="ly")
            nc.vector.tensor_tensor(Ly[:], Py[:], Fy[:], AluOp.subtract)
            Lx = co.tile([128, 2048], F32, tag="lx")
            nc.vector.tensor_tensor(Lx[:], Px[:], Fx[:], AluOp.subtract)
            Uy = co.tile([128, 2048], F32, tag="uy")
            nc.vector.tensor_scalar(Uy[:], Ly[:], -1.0, 1.0, AluOp.mult, AluOp.add)
            Ux = co.tile([128, 2048], F32, tag="ux")
            nc.vector.tensor_scalar(Ux[:], Lx[:], -1.0, 1.0, AluOp.mult, AluOp.add)
            W00 = co.tile([128, 2048], BF16, tag="w00")
            nc.vector.tensor_tensor(W00[:], Uy[:], Ux[:], AluOp.mult)
            W01 = co.tile([128, 2048], BF16, tag="w01")
            nc.vector.tensor_tensor(W01[:], Uy[:], Lx[:], AluOp.mult)
            W10 = co.tile([128, 2048], BF16, tag="w10")
            nc.vector.tensor_tensor(W10[:], Ly[:], Ux[:], AluOp.mult)
            W11 = co.tile([128, 2048], BF16, tag="w11")
            nc.vector.tensor_tensor(W11[:], Ly[:], Lx[:], AluOp.mult)
            Fcy = co.tile([128, 2048], F32, tag="py")
            nc.vector.tensor_scalar(
                Fcy[:], Fy[:], 512.0, float(512 + PH - 1), AluOp.max, AluOp.min
            )
            Fcx = co.tile([128, 2048], F32, tag="px")
            nc.vector.tensor_scalar(
                Fcx[:], Fx[:], 512.0, float(512 + PW - 1), AluOp.max, AluOp.min
            )
            Fmy = co.tile([128, 2048], F32, tag="uy")
            # Fmy = (Fcy-256)*131 - 256  (undo the +256 operand shift)
            nc.vector.tensor_scalar(
                Fmy[:], Fcy[:], float(PW), -float(512 * PW + 512), AluOp.mult, AluOp.add
            )
            I0 = co.tile([128, 2048], F32, tag="ux")
            nc.vector.tensor_tensor(I0[:], Fmy[:], Fcx[:], AluOp.add)
            I0i = co.tile([128, 2048], I16)
            nc.vector.tensor_copy(I0i[:], I0[:])

            # idx plane -> DRAM (partition-free staging for the wraps)
            istage = dp.tile([128, 2048], I16)
            nc.sync.dma_start(out=istage[:], in_=I0i[:])

            # ---- 3-6. per (tile, tap) ----
            for t in range(NT):
                g, s = t // 4, t % 4
                pout = ps_out.tile([O, NI], F32)
                for kk in range(TAPS):
                    row = 32 * g + kk
                    col = s * NI

                    # wrapped idx [128, NW] from DRAM staging: idx j of the
                    # gather sits at (partition j%16, col j//16); plane order
                    # makes this a contiguous repack, replicated to all cores
                    idxw = ip.tile([128, NW], I16)
                    iap = istage[:]
                    src = bass.AP(
                        tensor=iap.tensor,
                        offset=iap.offset + row * 2048 + col,
                        ap=[[0, 8], [NW, 16], [1, NW]],
                    )
                    nc.sync.dma_start(out=idxw[:], in_=src)

                    G = gp.tile([128, 2, NI], BF16)
                    nc.gpsimd.dma_gather(
                        G[:], table_d[:], idxw[:], NI, NI, 4 * C, transpose=True
                    )

                    # weight broadcast: pw[0:64,0:NI]=w00, [64:,0:NI]=w01,
                    # [0:64,NI:]=w10, [64:,NI:]=w11 (cols in gather order via
                    # phi(j) = (j%16)*32 + j//16)
                    pw = ps_w.tile([128, 2 * NI], F32)
                    lsel = sel[:, (9 * g + kk) * O : (9 * g + kk + 1) * O]
                    for pl, qd in ((W00, 0), (W01, 1), (W10, 2), (W11, 3)):
                        pap = pl[:, col : col + NI]
                        prhs = bass.AP(
                            tensor=pap.tensor,
                            offset=pap.offset,
                            ap=[pap.ap[0], [1, NW], [NW, 16]],
                        )
                        nc.tensor.matmul(
                            pw[
                                (qd % 2) * O : (qd % 2) * O + O,
                                (qd // 2) * NI : (qd // 2 + 1) * NI,
                            ],
                            lsel,
                            prhs,
                            start=True,
                            stop=True,
                        )
                    wi = wp.tile([128, 2 * NI], BF16)
                    nc.scalar.copy(wi[:], pw[:])

                    Sc = sp.tile([128, 2 * NI], BF16)
                    nc.vector.tensor_tensor(
                        Sc[:],
                        G[:].rearrange("p a b -> p (a b)"),
                        wi[:],
                        AluOp.mult,
                    )
                    nc.tensor.matmul(
                        pout[:], wdcn[:, kk * O : (kk + 1) * O], Sc[:, 0:NI],
                        start=(kk == 0), stop=False,
                    )
                    nc.tensor.matmul(
                        pout[:], wdcn[:, kk * O : (kk + 1) * O], Sc[:, NI : 2 * NI],
                        start=False, stop=(kk == TAPS - 1),
                    )

                of = op_.tile([O, NI], F32)
                nc.scalar.activation(
                    of[:], pout[:], mybir.ActivationFunctionType.Identity,
                    bias=bias[:],
                )
                nc.sync.dma_start(out=out_d[:, t * NI : (t + 1) * NI], in_=of[:])

    nc.compile()
    return nc


_PROGRAM = None


def _get_program():
    global _PROGRAM
    if _PROGRAM is None:
        _PROGRAM = _build_program()
    return _PROGRAM


def make_core_inputs(x, w_off, b_off, w_dcn, b_dcn):
    """Host-side prep: padded images, patch tables, base planes, weights."""
    x = np.asarray(x, dtype=np.float32)
    w_off = np.asarray(w_off, dtype=np.float32)
    b_off = np.asarray(b_off, dtype=np.float32)
    w_dcn = np.asarray(w_dcn, dtype=np.float32)
    b_dcn = np.asarray(b_dcn, dtype=np.float32)

    # padded fp32 image [B, C, PH, PW]: image at [2+y, 2+x]
    xpad = np.zeros((B, C, PH, PW), dtype=np.float32)
    xpad[:, :, 2 : 2 + H, 2 : 2 + W] = x

    # patch tables [B, NPOS, 4C] bf16 built from a (PH+1, PW+1) bf16 image
    xp16 = np.zeros((B, PH + 1, PW + 1, C), dtype=ml_dtypes.bfloat16)
    xp16[:, :PH, :PW] = xpad.transpose(0, 2, 3, 1).astype(ml_dtypes.bfloat16)
    tables = []
    for b in range(B):
        p = xp16[b]
        tab = np.concatenate(
            [
                p[:PH, :PW],
                p[:PH, 1 : PW + 1],
                p[1 : PH + 1, :PW],
                p[1 : PH + 1, 1 : PW + 1],
            ],
            axis=2,
        )  # [PH, PW, 4C]
        tables.append(np.ascontiguousarray(tab.reshape(NPOS, 4 * C)))

    # base planes per half [128, 2048]; tap row 32g+kk; free position f in
    # each 512 block holds pixel q(f) = (f%32)*16 + f//32
    f_idx = np.arange(NI)
    QPERM = (f_idx % 32) * 16 + f_idx // 32
    r_idx = np.arange(4)
    wo_idx = np.arange(W)
    basesy, basesx = [], []
    for h in range(2):
        bpy = np.zeros((128, 2048), dtype=np.float32)
        bpx = np.zeros((128, 2048), dtype=np.float32)
        for g in range(4):
            for kk in range(TAPS):
                ki, kj = kk // K, kk % K
                for s in range(4):
                    t = 4 * g + s
                    ho = 4 * t + r_idx + HALF * h  # [4]
                    by = (ho + ki + 1 + 512 + b_off[2 * kk])[:, None] + 0.0 * wo_idx[None, :]
                    bx = (wo_idx + kj + 1 + 512 + b_off[2 * kk + 1])[None, :] + 0.0 * r_idx[:, None]
                    bpy[32 * g + kk, s * NI : (s + 1) * NI] = by.reshape(-1)[QPERM]
                    bpx[32 * g + kk, s * NI : (s + 1) * NI] = bx.reshape(-1)[QPERM]
        basesy.append(bpy)
        basesx.append(bpx)

    # conv weights: lhsT per tap [C, MC]: dy taps cols 0-8, dx cols 32-40
    PERM_DY = [2 * t for t in range(TAPS)]
    PERM_DX = [2 * t + 1 for t in range(TAPS)]
    woff_l = np.zeros((C, TAPS * MC), dtype=np.float32)
    for kk in range(TAPS):
        ki, kj = kk // K, kk % K
        woff_l[:, kk * MC : kk * MC + 9] = w_off[PERM_DY, :, ki, kj].T
        woff_l[:, kk * MC + 32 : kk * MC + 41] = w_off[PERM_DX, :, ki, kj].T

    wdcn_l = np.zeros((2 * C, TAPS * O), dtype=ml_dtypes.bfloat16)
    for kk in range(TAPS):
        ki, kj = kk // K, kk % K
        wt = w_dcn[:, :, ki, kj].T.astype(ml_dtypes.bfloat16)  # [C, O]
        wdcn_l[0:C, kk * O : (kk + 1) * O] = wt
        wdcn_l[C : 2 * C, kk * O : (kk + 1) * O] = wt

    # selector lhsT [128, 36*64]: variant v = 9g+kk -> one-hot row 32g+kk
    sel = np.zeros((128, 36 * O), dtype=ml_dtypes.bfloat16)
    for g in range(4):
        for kk in range(TAPS):
            v = 9 * g + kk
            sel[32 * g + kk, v * O : (v + 1) * O] = 1.0
    bias = b_dcn.reshape(O, 1).astype(np.float32)

    in_maps = []
    for core in range(8):
        b, h = core // 2, core % 2
        xs = np.ascontiguousarray(
            xpad[b, :, h * HALF : h * HALF + XROWS, :].reshape(C, XROWS * PW)
        )
        in_maps.append(
            {
                "xs": xs,
                "table": tables[b],
                "basey": basesy[h],
                "basex": basesx[h],
                "woff": woff_l,
                "wdcn": wdcn_l,
                "sel": sel,
                "bias": bias,
            }
        )
    return in_maps


def kernel(x, w_off, b_off, w_dcn, b_dcn):
    nc = _get_program()
    in_maps = make_core_inputs(x, w_off, b_off, w_dcn, b_dcn)
    res = run_bass_kernel_spmd(nc, in_maps, core_ids=list(range(8)), trace=False)
    out = np.zeros((B, O, H, W), dtype=np.float32)
    for core in range(8):
        b, h = core // 2, core % 2
        out[b, :, h * HALF : (h + 1) * HALF, :] = res.results[core]["outc"].reshape(
            O, HALF, W
        )
    return out

